# revision 12
# baseline (speedup 1.0000x reference)
"""Trainium2 Bass kernel for AbsDiagNet.

Reference computation (T=256, B=128, I=512, H=2048, O=512):
    proj = einsum('tbi,hi->tbh', X, W_IH)
    h_0 = 0;  h_t = |proj_t + HH * h_{t-1}|   (elementwise over [B, H])
    Y = h_T @ W_HO.T + b_HO                   -> [B, O]

Strategy: data-parallel over batch across 8 cores (B_local = 16), params
replicated.  All operand transposes are done host-side during sharding so the
device only runs matmuls (bf16, full PE rate), the serial DVE recurrence,
and ACT-engine PSUM->SBUF copies.

Each recurrence step is ONE custom fused DVE op on a [128, 256] state tile
(h-on-partitions, (h_chunk, batch) on free dim):
    ABS_ADD:  h' = |h + proj_t|   (maxx(r, -r) with r = Src0+Src1; ~340ns/step
                                   measured vs ~715ns for add + sign-bit-and)

Per-core device pipeline over time segments (small head segments let the
serial DVE chain start early; small tail segments cut the end drain):
  one batched DMA per segment (all 4 i-chunks) -> PE: proj^T[h,(t,b)] bf16
  matmuls into bank-aligned 2-chunk PSUM tiles -> ACT: strided copy into a
  t-major SBUF proj buffer -> DVE: SEG fused recurrence steps.
W_HO^T loads late (only needed at the end); W_IH loads column-split so the
first h-chunks' matmuls can start after ~0.25MB.
Final: round h_T to f32r (ACT copy), then Y = h^T . W_HO^T + bias as a
17-matmul f32r PSUM accumulation group (bias folded in as a K=1 matmul
against a ones row).  Measured ~47-57us by the repeat-slope harness method
(baseline 198.7us same-method); output rel err ~2.4e-3 (bf16 proj GEMM,
fp32 recurrence, f32r output GEMM).
"""

import numpy as np
import ml_dtypes

import concourse.mybir as mybir
from concourse import bacc
from concourse.alu_op_type import AluOpType
from concourse.tile import TileContext
from concourse.bass_utils import run_bass_kernel_spmd

import concourse.dve_ops as dve_ops
from concourse.dve_ops import DveOp
from concourse.dve_spec import Spec, Src0, Src1, maxx, lower
from concourse.dve_uop import DveOpSpec

# Problem shape (hardcoded per contract).
T, B, I, H, O = 256, 128, 512, 2048, 512
NCORES = 8
BL = B // NCORES            # 16 batch elements per core
NC_H = H // 128             # 16 h-chunks
NC_I = I // 128             # 4 i-chunks
CPAIR = 2                   # h-chunks per PSUM tile (2 banks)
F32 = mybir.dt.float32
F32R = mybir.dt.float32r
BF16 = mybir.dt.bfloat16
I32 = mybir.dt.int32
BF_NP = ml_dtypes.bfloat16

# time segments: small head so the serial DVE chain starts early; small
# tail so the post-PE recurrence drain is short
SEGS = [4, 8, 20, 32, 32, 32, 32, 32, 32, 16, 12, 4]
assert sum(SEGS) == T


def _register_abs_add():
    """Fused custom DVE op: out = |in0 + in1| (one instruction per
    recurrence step).  Registered at import so kernel.py stays
    self-contained; the sha is computed by lowering the spec."""
    if "ABS_ADD_ANT" in dve_ops._SUB_OPCODE_FOR_NAME:
        return next(o for o in dve_ops.OPS if o.name == "ABS_ADD_ANT")
    r = Src0 + Src1
    spec = Spec(
        body=maxx(r, -r),
        reference=lambda in0, in1, s0, s1, imm2: np.abs(
            in0.astype(np.float32) + in1.astype(np.float32)
        ),
    )
    name = "ABS_ADD_ANT"
    opcode = dve_ops._CUSTOM_DVE_ROW_BASE + len(dve_ops.OPS)
    assert opcode < 0x20, "custom DVE opcode row overflow"
    shas = {}
    for ver in ("v3",):
        s = DveOpSpec(name=name, opcode=opcode, uops=lower(spec, ver=ver),
                      rd1_en=True)
        shas[ver] = s.sha(ver)
    op = DveOp(name, spec, subdim=False, uops_sha=shas)
    dve_ops.OPS.append(op)
    dve_ops._SUB_OPCODE_FOR_NAME[name] = opcode
    dve_ops.CUSTOM_DVE_SPECS[name] = spec
    return op


ABS_ADD = _register_abs_add()


def _build(apply_hh: bool, repeat: int = 1):
    nc = bacc.Bacc("TRN2", target_bir_lowering=False, debug=False)

    xt = nc.dram_tensor("xt", [I, T * BL], BF16, kind="ExternalInput")
    wih_t = nc.dram_tensor("wih_t", [I, H], BF16, kind="ExternalInput")
    who_t = nc.dram_tensor("who_t", [H, O], F32R, kind="ExternalInput")
    bias = nc.dram_tensor("bias", [1, O], F32R, kind="ExternalInput")
    ones = nc.dram_tensor("ones", [1, BL], F32R, kind="ExternalInput")
    if apply_hh:
        hhb = nc.dram_tensor("hhb", [128, NC_H * BL], F32, kind="ExternalInput")
    y = nc.dram_tensor("y", [BL, O], F32, kind="ExternalOutput")

    xt3 = xt.rearrange("(ic p) f -> p ic f", ic=NC_I, p=128)
    who3 = who_t.rearrange("(c p) f -> p c f", c=NC_H, p=128)

    with TileContext(nc) as tc:
        with (
            tc.tile_pool(name="wpool", bufs=1) as wpool,
            tc.tile_pool(name="xpool", bufs=4) as xpool,
            tc.tile_pool(name="ppool", bufs=2) as ppool,
            tc.tile_pool(name="spool", bufs=1) as spool,
            tc.tile_pool(name="psum", bufs=4, space="PSUM") as psum,
        ):
            # --- prefetch first xt block before weights ---
            xq = []
            xtile0 = xpool.tile([128, NC_I * SEGS[0] * BL], BF16, tag="xt0")
            nc.sync.dma_start(
                out=xtile0.rearrange("p (ic f) -> p ic f", ic=NC_I),
                in_=xt3[:, :, 0:SEGS[0] * BL],
            )
            xq.append(xtile0)
            # --- weights, bias, constants (resident) ---
            # wih loads split column-wise so the first h-chunks arrive fast
            HSPLIT = 256
            wih_sb = []
            for ic in range(NC_I):
                w = wpool.tile([128, H], BF16, tag=f"wih{ic}")
                nc.sync.dma_start(
                    out=w[:, :HSPLIT],
                    in_=wih_t[ic * 128:(ic + 1) * 128, :HSPLIT],
                )
                wih_sb.append(w)
            for ic in range(NC_I):
                nc.sync.dma_start(
                    out=wih_sb[ic][:, HSPLIT:],
                    in_=wih_t[ic * 128:(ic + 1) * 128, HSPLIT:],
                )
            bias_sb = wpool.tile([1, O], F32R, tag="bias")
            nc.sync.dma_start(out=bias_sb, in_=bias[:, :])
            ones_sb = wpool.tile([1, BL], F32R, tag="ones")
            nc.sync.dma_start(out=ones_sb, in_=ones[:, :])
            whot_sb = wpool.tile([128, NC_H * O], F32R, tag="whot")
            whot_loaded = False
            if apply_hh:
                hhb_sb = wpool.tile([128, NC_H * BL], F32, tag="hhb")
                nc.sync.dma_start(out=hhb_sb, in_=hhb[:, :])

            # --- recurrence state: [128, (c, b)]; h = c*128 + p ---
            sA = spool.tile([128, NC_H * BL], F32, tag="sA")
            sB = spool.tile([128, NC_H * BL], F32, tag="sB")

            for _rep in range(repeat):
                if apply_hh:
                    nc.vector.memset(sA, 0.0)
                states = [sA, sB]
                t0seg = 0
                for si, SEG in enumerate(SEGS):
                    tb0 = t0seg
                    t0seg += SEG
                    # proj buffer, t-major: free index = t*256 + c*16 + b
                    proj = ppool.tile([128, SEG * NC_H * BL], F32, tag="proj")
                    proj3 = proj.rearrange(
                        "p (t cb) -> p t cb", t=SEG, cb=NC_H * BL
                    )
                    if xq:
                        xtile = xq.pop(0)
                    else:
                        xtile = xpool.tile([128, NC_I * SEG * BL], BF16, tag="xt")
                        nc.sync.dma_start(
                            out=xtile.rearrange("p (ic f) -> p ic f", ic=NC_I),
                            in_=xt3[:, :, tb0 * BL:(tb0 + SEG) * BL],
                        )
                    xtile3 = xtile.rearrange("p (ic f) -> p ic f", ic=NC_I)
                    if si == len(SEGS) - 3 and not whot_loaded:
                        # load the output weights late, off the critical path
                        nc.sync.dma_start(
                            out=whot_sb.rearrange("p (c f) -> p c f", c=NC_H),
                            in_=who3,
                        )
                        whot_loaded = True
                    for cp in range(NC_H // CPAIR):
                        # one full bank (512 fp32) per cc so each slice is
                        # bank-aligned even when SEG*BL < 512
                        ps = psum.tile([128, CPAIR * 512], F32, tag="mm")
                        for cc in range(CPAIR):
                            c = cp * CPAIR + cc
                            for ic in range(NC_I):
                                nc.tensor.matmul(
                                    out=ps[:, cc * 512:cc * 512 + SEG * BL],
                                    lhsT=wih_sb[ic][:, c * 128:(c + 1) * 128],
                                    rhs=xtile3[:, ic, :],
                                    start=(ic == 0),
                                    stop=(ic == NC_I - 1),
                                )
                        # PSUM [128, (cc, t, b)] -> SBUF t-major slice
                        nc.scalar.copy(
                            out=proj3[
                                :, :, cp * CPAIR * BL:(cp + 1) * CPAIR * BL
                            ].rearrange("p t (cc b) -> p t cc b", cc=CPAIR, b=BL),
                            in_=ps.rearrange("p (cc f) -> p cc f", cc=CPAIR)[
                                :, :, :SEG * BL
                            ].rearrange(
                                "p cc (t b) -> p cc t b", t=SEG, b=BL
                            ).transpose([0, 2, 1, 3]),
                        )
                    for tl in range(SEG):
                        src, dst = states
                        if apply_hh:
                            # general path: h' = |hh*h + p|
                            nc.vector.tensor_mul(out=dst, in0=src, in1=hhb_sb)
                            nc.vector.tensor_add(
                                out=dst, in0=dst, in1=proj3[:, tl, :]
                            )
                            nc.vector.tensor_scalar(
                                out=src.bitcast(I32), in0=dst.bitcast(I32),
                                scalar1=0x7FFFFFFF, scalar2=None,
                                op0=AluOpType.bitwise_and,
                            )
                            states = [src, dst]
                        elif si == 0 and tl == 0:
                            # h_1 = |0 + p_0| = |p_0|: sign-bit clear straight
                            # from proj; skips the h memset and its chain-start
                            # dependency
                            nc.vector.tensor_scalar(
                                out=dst.bitcast(I32),
                                in0=proj3[:, 0, :].bitcast(I32),
                                scalar1=0x7FFFFFFF, scalar2=None,
                                op0=AluOpType.bitwise_and,
                            )
                            states = [dst, src]
                        else:
                            # h' = |h + p| as one fused DVE instruction
                            # (two interleaved half-width chains measured
                            # worse: +256 instructions of dispatch outweigh
                            # the write-drain hiding)
                            nc.vector._custom_dve(
                                ABS_ADD, out=dst, in0=proj3[:, tl, :], in1=src,
                            )
                            states = [dst, src]

                # round h_T to f32r for the output matmul (ACT engine,
                # off the DVE critical path)
                rfin = states[0]
                sAr = spool.tile([128, NC_H * BL], F32R, tag="sar")
                nc.scalar.copy(out=sAr, in_=rfin)
                sA3 = sAr.rearrange("p (c b) -> p c b", c=NC_H, b=BL)
                yps = psum.tile([BL, O], F32, tag="mm")
                for c in range(NC_H):
                    nc.tensor.matmul(
                        out=yps,
                        lhsT=sA3[:, c, :],
                        rhs=whot_sb[:, c * O:(c + 1) * O],
                        start=(c == 0),
                        stop=False,
                    )
                nc.tensor.matmul(
                    out=yps, lhsT=ones_sb, rhs=bias_sb, start=False, stop=True,
                )
                y_sb = spool.tile([BL, O], F32, tag="y")
                nc.scalar.copy(out=y_sb, in_=yps)
                nc.sync.dma_start(out=y[:, :], in_=y_sb)

    nc.compile()
    return nc


def prep_inputs(X, W_IH, W_HO, b_HO):
    """Host-side prep shared by kernel() and test.py: transposes + bf16
    casts + per-core X shards."""
    wih_t = np.ascontiguousarray(W_IH.T).astype(BF_NP)       # [I, H]
    who_t = np.ascontiguousarray(W_HO.T).astype(np.float32)  # [H, O]
    common = {"wih_t": wih_t, "who_t": who_t,
              "bias": b_HO.reshape(1, O).astype(np.float32),
              "ones": np.ones((1, BL), dtype=np.float32)}
    xts = []
    for k in range(NCORES):
        xk = X[:, k * BL:(k + 1) * BL, :]                    # [T, BL, I]
        xt = np.ascontiguousarray(
            xk.transpose(2, 0, 1)
        ).reshape(I, T * BL).astype(BF_NP)
        xts.append(xt)
    return common, xts


def kernel(X, W_IH, HH, W_HO, b_HO, _cache={}):
    X = np.asarray(X, dtype=np.float32)
    W_IH = np.asarray(W_IH, dtype=np.float32)
    HH = np.asarray(HH, dtype=np.float32)
    W_HO = np.asarray(W_HO, dtype=np.float32)
    b_HO = np.asarray(b_HO, dtype=np.float32)

    apply_hh = not np.all(HH == 1.0)

    if ("nc", apply_hh) not in _cache:
        _cache[("nc", apply_hh)] = _build(apply_hh)
    nc = _cache[("nc", apply_hh)]

    common, xts = prep_inputs(X, W_IH, W_HO, b_HO)
    if apply_hh:
        # hhb[p, c*BL + b] = HH[c*128 + p]
        hhb = np.repeat(
            HH.reshape(NC_H, 128).T[:, :, None], BL, axis=2
        ).reshape(128, NC_H * BL)
        common["hhb"] = np.ascontiguousarray(hhb)

    in_maps = [{"xt": xts[k], **common} for k in range(NCORES)]

    res = run_bass_kernel_spmd(nc, in_maps, core_ids=list(range(NCORES)))
    out = np.concatenate([res.results[k]["y"] for k in range(NCORES)], axis=0)
    return out.astype(np.float32)
